# revision 10
# baseline (speedup 1.0000x reference)
"""Trainium2 Bass kernel for nn_BinaryLinear (8192x4096 @ sign(4096x4096).T + BN + sign).

Math: out = sign((y - mean_b(y)) * rsqrt(var + eps) * gamma + beta), y = x @ sign(W).T + b.
With the reference's gamma == 1 (> 0) and beta == 0 the rsqrt/gamma factor is a positive
per-channel scale and beta vanishes, so out == sign(y - mean_b(y)); the bias b cancels in
y - mean, and mean is linear in x: y - mean = x @ Wb.T - m with m = colmean(x) @ Wb.T.
The kernel computes out = sign(x @ Wb.T - m) — no variance pass, no second sweep over y.

Distribution: data-parallel over the batch dim, 1024 rows per NeuronCore.  Per-channel
column sums of x are AllReduced (16 KB) so every core subtracts the same global mean.

Precision: the matmul runs as two bf16 passes (x_hi + x_lo, x = x_hi + x_lo exact to
~2^-18 relative) accumulated in fp32 PSUM.  Wb = sign(W) is exact in bf16, so the result
matches an fp32 matmul to ~1e-6 relative — only a handful of sign flips at
|y - mean| ~ 1e-4 remain vs the fp32 reference.

Per-core schedule:
  * W pipeline (from t=0): stream W rows, Sign on ACT -> bf16 Wb, bounce to DRAM.
  * P1: split x into bf16 hi/lo (DRAM bounce, per half-batch tile) + ones-matmul column
    sums; AllReduce; broadcast -colmean(x) to a [128, D] fp32 tile.
  * m: per o-tile, reload Wb rows and fold with the broadcast -xbar on the vector engine
    (tensor_tensor_reduce) -> bias tile "-m" (exact fp32 accumulation, zero PE cost).
  * Main loop over 32 o-tiles: xbar-transposed Wb load -> lhsT [i, o]; 32 k-steps of
    4 N=512 matmuls (hi/lo x two batch halves) sharing one weight load; Sign epilogue
    with bias=-m straight from PSUM on ACT; one xbar transpose back to [b, o] layout and
    one SWDGE store that casts bf16 -> fp32 on the fly.
x loads + all xbar transposes issue on SP, stores + W pipeline on SWDGE to keep DMA
issue queues from serializing the weight prefetch.
"""
import sys

try:
    import concourse.bass as bass
except ImportError:
    sys.path.insert(0, "/opt/trn_rl_repo")
    import concourse.bass as bass

import numpy as np
import concourse.mybir as mybir
import concourse.tile as tile
from concourse import bacc
from concourse.bass_utils import run_bass_kernel_spmd

N_CORES = 8
B, D = 8192, 4096
BS = B // N_CORES          # 1024 batch rows per core
P = 128
NB = BS // P               # 8 batch tiles per core
NK = D // P                # 32 contraction tiles
NO = D // P                # 32 output-channel tiles
HB = BS // 2               # 512 rows per batch-half
QD = D // 4                # W processed in quarter-rows
F32, BF16 = mybir.dt.float32, mybir.dt.bfloat16

_CACHED_NC = None


def _build_nc():
    nc = bacc.Bacc("TRN2", target_bir_lowering=False, debug=False, num_devices=N_CORES)
    xs = nc.declare_dram_parameter("xs", [BS, D], F32, isOutput=False)
    W = nc.declare_dram_parameter("W", [D, D], F32, isOutput=False)
    out = nc.declare_dram_parameter("out", [BS, D], F32, isOutput=True)

    with tile.TileContext(nc) as tc:
        with (
            tc.tile_pool(name="const", bufs=1) as const,
            tc.tile_pool(name="stats", bufs=1) as stats,
            tc.tile_pool(name="wstage", bufs=3) as wstage,
            tc.tile_pool(name="wsign", bufs=3) as wsign,
            tc.tile_pool(name="wT", bufs=2) as wTp,
            tc.tile_pool(name="epi", bufs=2) as epi,
            tc.tile_pool(name="ps", bufs=2, space="PSUM") as ps,
            tc.tile_pool(name="pttr", bufs=1, space="PSUM") as pttr,
            tc.tile_pool(name="dram", bufs=1, space="DRAM") as dram,
            tc.tile_pool(name="wbdram", bufs=NO, space="DRAM") as wbdram,
        ):
            ones = const.tile([P, 1], BF16)
            nc.vector.memset(ones[:], 1.0)
            xbar_bc = stats.tile([P, D], F32)   # broadcast -colmean(x), filled later

            xhi_d = [dram.tile([HB, D], BF16, tag=f"xhi{h}", name=f"xhi{h}")
                     for h in range(2)]
            xlo_d = [dram.tile([HB, D], BF16, tag=f"xlo{h}", name=f"xlo{h}")
                     for h in range(2)]
            cs_in = dram.tile([1, D], F32, tag="cs_in")
            cs_out = dram.tile([1, D], F32, tag="cs_out")

            # ---- P1 (scoped pools): split x, store hi/lo, column sums
            with (
                tc.tile_pool(name="xstage", bufs=2) as xstage,
                tc.tile_pool(name="xsplit", bufs=2) as xsplit,
                tc.tile_pool(name="csacc", bufs=1) as csacc,
                tc.tile_pool(name="pcs", bufs=2, space="PSUM") as pcsp,
            ):
                NCH = D // 512
                cs_chunks = []
                for c in range(NCH):
                    t = csacc.tile([1, 512], F32, tag=f"cs{c}", name=f"cs{c}")
                    nc.vector.memset(t[:], 0.0)
                    cs_chunks.append(t)
                HD = D // 2
                for bt in range(NB):
                    half, row = bt // (NB // 2), (bt % (NB // 2)) * P
                    for hh in range(2):
                        c0 = hh * HD
                        xf = xstage.tile([P, HD], F32, tag="xf", name=f"xf{bt}_{hh}")
                        nc.sync.dma_start(xf[:], xs[bt * P:(bt + 1) * P, c0:c0 + HD])
                        xh = xsplit.tile([P, HD], BF16, tag="xh", name=f"xh{bt}_{hh}")
                        nc.vector.tensor_copy(xh[:], xf[:])
                        nc.vector.tensor_sub(xf[:], xf[:], xh[:])
                        xl = xsplit.tile([P, HD], BF16, tag="xl", name=f"xl{bt}_{hh}")
                        nc.vector.tensor_copy(xl[:], xf[:])
                        nc.gpsimd.dma_start(
                            xhi_d[half][row:row + P, c0:c0 + HD], xh[:])
                        nc.gpsimd.dma_start(
                            xlo_d[half][row:row + P, c0:c0 + HD], xl[:])
                        for c in range(HD // 512):
                            g = hh * (HD // 512) + c
                            pcs = pcsp.tile([1, 512], F32, tag="pcs",
                                            name=f"pcs{bt}_{hh}_{c}")
                            nc.tensor.matmul(pcs[:], ones[:],
                                             xh[:, c * 512:(c + 1) * 512],
                                             start=True, stop=False)
                            nc.tensor.matmul(pcs[:], ones[:],
                                             xl[:, c * 512:(c + 1) * 512],
                                             start=False, stop=True)
                            nc.vector.tensor_add(cs_chunks[g][:], cs_chunks[g][:],
                                                 pcs[:])
                for c in range(NCH):
                    nc.gpsimd.dma_start(cs_in[0:1, c * 512:(c + 1) * 512],
                                        cs_chunks[c][:])

            # ---- W pipeline: binarize W rows to bf16 and bounce to DRAM
            # (emitted after P1 so the x path keeps DMA priority; overlaps main loop)
            wb_tiles = []
            for o in range(NO):
                wb_d = wbdram.tile([P, D], BF16, tag="wb", name=f"wb{o}")
                wb_tiles.append(wb_d)
                for q in range(4):
                    wf = wstage.tile([P, QD], F32, tag="wf", name=f"wf{o}_{q}")
                    nc.gpsimd.dma_start(wf[:], W[o * P:(o + 1) * P, q * QD:(q + 1) * QD])
                    wsg = wsign.tile([P, QD], BF16, tag="ws", name=f"ws{o}_{q}")
                    nc.scalar.sign(wsg[:], wf[:])
                    nc.gpsimd.dma_start(wb_d[:, q * QD:(q + 1) * QD], wsg[:])

            # ---- AllReduce colsum; build broadcast -xbar tile
            nc.gpsimd.collective_compute(
                "AllReduce", mybir.AluOpType.add,
                replica_groups=[list(range(N_CORES))],
                ins=[cs_in.opt()], outs=[cs_out.opt()],
            )
            bc_src = bass.AP(tensor=cs_out[:].tensor, offset=cs_out[:].offset,
                             ap=[[0, P]] + list(cs_out[:].ap[1:]))
            nc.gpsimd.dma_start(xbar_bc[:], bc_src)
            nc.vector.tensor_scalar_mul(xbar_bc[:], xbar_bc[:], -1.0 / B)

            # ---- -m per o-tile: fold Wb rows with -xbar on DVE (exact fp32 accum)
            negm_tiles = []
            for o in range(NO):
                negm = stats.tile([P, 1], F32, tag=f"negm{o}", name=f"negm{o}")
                negm_tiles.append(negm)
                parts = []
                for q in range(4):
                    wbr = wsign.tile([P, QD], BF16, tag="ws", name=f"wbr{o}_{q}")
                    nc.sync.dma_start(wbr[:], wb_tiles[o][:, q * QD:(q + 1) * QD])
                    prod = pttr.tile([P, QD], F32, tag="prod", name=f"prod{o}_{q}")
                    nc.vector.tensor_tensor(
                        out=prod[:], in0=wbr[:],
                        in1=xbar_bc[:, q * QD:(q + 1) * QD],
                        op=mybir.AluOpType.mult)
                    mp = stats.tile([P, 1], F32, tag=f"mp{o}_{q}", name=f"mp{o}_{q}")
                    nc.vector.reduce_sum(mp[:], prod[:], axis=mybir.AxisListType.X)
                    parts.append(mp)
                nc.vector.tensor_add(negm[:], parts[0][:], parts[1][:])
                nc.vector.tensor_add(negm[:], negm[:], parts[2][:])
                nc.vector.tensor_add(negm[:], negm[:], parts[3][:])

            # ---- main loop (xT pool opens after P1 pools closed)
            with tc.tile_pool(name="xT", bufs=1) as xTp:
                xT = {}
                for k in range(NK):
                    for h in range(2):
                        th = xTp.tile([P, HB], BF16, tag=f"xh{k}h{h}",
                                      name=f"xTh{k}_{h}")
                        nc.sync.dma_start_transpose(
                            th[:], xhi_d[h][:, k * P:(k + 1) * P])
                        tl = xTp.tile([P, HB], BF16, tag=f"xl{k}h{h}",
                                      name=f"xTl{k}_{h}")
                        nc.sync.dma_start_transpose(
                            tl[:], xlo_d[h][:, k * P:(k + 1) * P])
                        xT[("h", k, h)] = th
                        xT[("l", k, h)] = tl

                for o in range(NO):
                    wT = wTp.tile([P, NK, P], BF16, tag="wT", name=f"wT{o}")
                    nc.sync.dma_start_transpose(wT[:], wb_tiles[o][:, :])
                    psum = ps.tile([P, BS], F32, tag="acc", name=f"acc{o}")
                    for k in range(NK):
                        lhsT = wT[:, k, :]
                        nc.tensor.matmul(psum[:, 0:HB], lhsT, xT[("h", k, 0)][:],
                                         start=(k == 0), stop=False)
                        nc.tensor.matmul(psum[:, HB:BS], lhsT, xT[("h", k, 1)][:],
                                         start=(k == 0), stop=False)
                        nc.tensor.matmul(psum[:, 0:HB], lhsT, xT[("l", k, 0)][:],
                                         start=False, stop=(k == NK - 1))
                        nc.tensor.matmul(psum[:, HB:BS], lhsT, xT[("l", k, 1)][:],
                                         start=False, stop=(k == NK - 1))
                    ys = epi.tile([P, BS], BF16, tag="ys", name=f"ys{o}")
                    nc.scalar.activation(out=ys[:], in_=psum[:],
                                         func=mybir.ActivationFunctionType.Sign,
                                         bias=negm_tiles[o][:], scale=1.0)
                    ysT = epi.tile([P, NB, P], BF16, tag="ysT", name=f"ysT{o}")
                    nc.sync.dma_start_transpose(ysT[:], ys[:])
                    ysT32 = epi.tile([P, NB, P], F32, tag="ysT32", name=f"ysT32{o}")
                    nc.vector.tensor_copy(ysT32[:], ysT[:])
                    nc.sync.dma_start(
                        out[:, o * P:(o + 1) * P].rearrange("(t p) j -> p t j", p=P),
                        ysT32[:])

    nc.finalize()
    return nc


def _get_nc():
    global _CACHED_NC
    if _CACHED_NC is None:
        _CACHED_NC = _build_nc()
    return _CACHED_NC


def _run(x, W, **kw):
    nc = _get_nc()
    in_maps = [{"xs": x[c * BS:(c + 1) * BS], "W": W} for c in range(N_CORES)]
    res = run_bass_kernel_spmd(nc, in_maps, list(range(N_CORES)), **kw)
    full = np.concatenate([res.results[c]["out"] for c in range(N_CORES)], axis=0)
    return full, res


def kernel(x, W, b, gamma, beta):
    x = np.ascontiguousarray(x, dtype=np.float32)
    W = np.ascontiguousarray(W, dtype=np.float32)
    assert x.shape == (B, D) and W.shape == (D, D)
    if not (np.all(np.asarray(gamma) > 0) and np.all(np.asarray(beta) == 0)):
        # The sign(y - mean) reduction needs gamma > 0 and beta == 0 (always true for
        # this problem's inputs).  Otherwise fall back to a host computation.
        Wb = np.sign(W)
        y = x @ Wb.T + np.asarray(b, np.float32)
        mean = y.mean(0)
        var = ((y - mean) ** 2).mean(0)
        yn = (y - mean) / np.sqrt(var + 1e-5) * np.asarray(gamma) + np.asarray(beta)
        return np.sign(yn).astype(np.float32)
    full, _ = _run(x, W)
    return full.astype(np.float32, copy=False)


# revision 11
# speedup vs baseline: 1.2456x; 1.2456x over previous
"""Trainium2 Bass kernel for nn_BinaryLinear (8192x4096 @ sign(4096x4096).T + BN + sign).

Math: out = sign((y - mean_b(y)) * rsqrt(var + eps) * gamma + beta), y = x @ sign(W).T + b.
With the reference's gamma == 1 (> 0) and beta == 0 the rsqrt/gamma factor is a positive
per-channel scale and beta vanishes, so out == sign(y - mean_b(y)); the bias b cancels in
y - mean, and mean is linear in x: y - mean = x @ Wb.T - m with m = colmean(x) @ Wb.T.
The kernel computes out = sign(x @ Wb.T - m) — no variance pass, no second sweep over y.

Distribution: data-parallel over the batch dim, 1024 rows per NeuronCore.  Per-channel
column sums of x are AllReduced (16 KB) so every core subtracts the same global mean.

Precision: the matmul runs as two bf16 passes (x_hi + x_lo, x = x_hi + x_lo exact to
~2^-18 relative) accumulated in fp32 PSUM.  Wb = sign(W) is exact in bf16, so the result
matches an fp32 matmul to ~1e-6 relative — only a handful of sign flips at
|y - mean| ~ 1e-4 remain vs the fp32 reference.

Per-core schedule:
  * W pipeline (from t=0): stream W rows, Sign on ACT -> bf16 Wb, bounce to DRAM.
  * P1: split x into bf16 hi/lo (DRAM bounce, per half-batch tile) + ones-matmul column
    sums; AllReduce; broadcast -colmean(x) to a [128, D] fp32 tile.
  * m: per o-tile, reload Wb rows and fold with the broadcast -xbar on the vector engine
    (tensor_tensor_reduce) -> bias tile "-m" (exact fp32 accumulation, zero PE cost).
  * Main loop over 32 o-tiles: xbar-transposed Wb load -> lhsT [i, o]; 32 k-steps of
    4 N=512 matmuls (hi/lo x two batch halves) sharing one weight load; Sign epilogue
    with bias=-m straight from PSUM on ACT; one xbar transpose back to [b, o] layout and
    one SWDGE store that casts bf16 -> fp32 on the fly.
x loads + all xbar transposes issue on SP, stores + W pipeline on SWDGE to keep DMA
issue queues from serializing the weight prefetch.
"""
import sys

try:
    import concourse.bass as bass
except ImportError:
    sys.path.insert(0, "/opt/trn_rl_repo")
    import concourse.bass as bass

import numpy as np
import concourse.mybir as mybir
import concourse.tile as tile
from concourse import bacc
from concourse.bass_utils import run_bass_kernel_spmd

N_CORES = 8
B, D = 8192, 4096
BS = B // N_CORES          # 1024 batch rows per core
P = 128
NB = BS // P               # 8 batch tiles per core
NK = D // P                # 32 contraction tiles
NO = D // P                # 32 output-channel tiles
HB = BS // 2               # 512 rows per batch-half
QD = D // 4                # W processed in quarter-rows
F32, BF16 = mybir.dt.float32, mybir.dt.bfloat16

_CACHED_NC = None


def _build_nc():
    nc = bacc.Bacc("TRN2", target_bir_lowering=False, debug=False, num_devices=N_CORES)
    xs = nc.declare_dram_parameter("xs", [BS, D], F32, isOutput=False)
    W = nc.declare_dram_parameter("W", [D, D], F32, isOutput=False)
    out = nc.declare_dram_parameter("out", [BS, D], F32, isOutput=True)

    with tile.TileContext(nc) as tc:
        with (
            tc.tile_pool(name="const", bufs=1) as const,
            tc.tile_pool(name="stats", bufs=1) as stats,
            tc.tile_pool(name="wstage", bufs=3) as wstage,
            tc.tile_pool(name="wsign", bufs=3) as wsign,
            tc.tile_pool(name="wT", bufs=2) as wTp,
            tc.tile_pool(name="epi", bufs=2) as epi,
            tc.tile_pool(name="ps", bufs=2, space="PSUM") as ps,
            tc.tile_pool(name="pttr", bufs=1, space="PSUM") as pttr,
            tc.tile_pool(name="dram", bufs=1, space="DRAM") as dram,
            tc.tile_pool(name="wbdram", bufs=NO, space="DRAM") as wbdram,
        ):
            ones = const.tile([P, 1], BF16)
            nc.vector.memset(ones[:], 1.0)
            xbar_bc = stats.tile([P, D], F32)   # broadcast -colmean(x), filled later

            xhi_d = [dram.tile([HB, D], BF16, tag=f"xhi{h}", name=f"xhi{h}")
                     for h in range(2)]
            xlo_d = [dram.tile([HB, D], BF16, tag=f"xlo{h}", name=f"xlo{h}")
                     for h in range(2)]
            cs_in = dram.tile([1, D], F32, tag="cs_in")
            cs_out = dram.tile([1, D], F32, tag="cs_out")

            # ---- P1 (scoped pools): split x, store hi/lo, column sums
            with (
                tc.tile_pool(name="xstage", bufs=2) as xstage,
                tc.tile_pool(name="xsplit", bufs=2) as xsplit,
                tc.tile_pool(name="csacc", bufs=1) as csacc,
                tc.tile_pool(name="pcs", bufs=2, space="PSUM") as pcsp,
            ):
                NCH = D // 512
                cs_chunks = []
                for c in range(NCH):
                    t = csacc.tile([1, 512], F32, tag=f"cs{c}", name=f"cs{c}")
                    nc.vector.memset(t[:], 0.0)
                    cs_chunks.append(t)
                HD = D // 2
                for bt in range(NB):
                    half, row = bt // (NB // 2), (bt % (NB // 2)) * P
                    for hh in range(2):
                        c0 = hh * HD
                        xf = xstage.tile([P, HD], F32, tag="xf", name=f"xf{bt}_{hh}")
                        nc.sync.dma_start(xf[:], xs[bt * P:(bt + 1) * P, c0:c0 + HD])
                        xh = xsplit.tile([P, HD], BF16, tag="xh", name=f"xh{bt}_{hh}")
                        nc.vector.tensor_copy(xh[:], xf[:])
                        nc.vector.tensor_sub(xf[:], xf[:], xh[:])
                        xl = xsplit.tile([P, HD], BF16, tag="xl", name=f"xl{bt}_{hh}")
                        nc.vector.tensor_copy(xl[:], xf[:])
                        nc.gpsimd.dma_start(
                            xhi_d[half][row:row + P, c0:c0 + HD], xh[:])
                        nc.gpsimd.dma_start(
                            xlo_d[half][row:row + P, c0:c0 + HD], xl[:])
                        for c in range(HD // 512):
                            g = hh * (HD // 512) + c
                            pcs = pcsp.tile([1, 512], F32, tag="pcs",
                                            name=f"pcs{bt}_{hh}_{c}")
                            nc.tensor.matmul(pcs[:], ones[:],
                                             xh[:, c * 512:(c + 1) * 512],
                                             start=True, stop=False)
                            nc.tensor.matmul(pcs[:], ones[:],
                                             xl[:, c * 512:(c + 1) * 512],
                                             start=False, stop=True)
                            nc.vector.tensor_add(cs_chunks[g][:], cs_chunks[g][:],
                                                 pcs[:])
                for c in range(NCH):
                    nc.gpsimd.dma_start(cs_in[0:1, c * 512:(c + 1) * 512],
                                        cs_chunks[c][:])

            # ---- AllReduce colsum; build broadcast -xbar tile
            nc.gpsimd.collective_compute(
                "AllReduce", mybir.AluOpType.add,
                replica_groups=[list(range(N_CORES))],
                ins=[cs_in.opt()], outs=[cs_out.opt()],
            )
            bc_src = bass.AP(tensor=cs_out[:].tensor, offset=cs_out[:].offset,
                             ap=[[0, P]] + list(cs_out[:].ap[1:]))
            nc.gpsimd.dma_start(xbar_bc[:], bc_src)
            nc.vector.tensor_scalar_mul(xbar_bc[:], xbar_bc[:], -1.0 / B)

            # ---- W pipeline: binarize W rows to bf16, bounce to DRAM, and fold
            # each signed piece with -xbar on DVE for the -m bias (exact fp32
            # accumulation).  Gated on xbar (post-AllReduce) so the x path gets
            # DMA priority first; production stays well ahead of the main loop.
            wb_tiles = []
            negm_tiles = []
            for o in range(NO):
                wb_d = wbdram.tile([P, D], BF16, tag="wb", name=f"wb{o}")
                wb_tiles.append(wb_d)
                negm = stats.tile([P, 1], F32, tag=f"negm{o}", name=f"negm{o}")
                negm_tiles.append(negm)
                parts = []
                for q in range(4):
                    wf = wstage.tile([P, QD], F32, tag="wf", name=f"wf{o}_{q}")
                    nc.sync.dma_start(wf[:], W[o * P:(o + 1) * P, q * QD:(q + 1) * QD])
                    wsg = wsign.tile([P, QD], BF16, tag="ws", name=f"ws{o}_{q}")
                    nc.scalar.sign(wsg[:], wf[:])
                    nc.gpsimd.dma_start(wb_d[:, q * QD:(q + 1) * QD], wsg[:])
                    prod = pttr.tile([P, QD], F32, tag="prod", name=f"prod{o}_{q}")
                    nc.vector.tensor_tensor(
                        out=prod[:], in0=wsg[:],
                        in1=xbar_bc[:, q * QD:(q + 1) * QD],
                        op=mybir.AluOpType.mult)
                    mp = stats.tile([P, 1], F32, tag=f"mp{o}_{q}", name=f"mp{o}_{q}")
                    nc.vector.reduce_sum(mp[:], prod[:], axis=mybir.AxisListType.X)
                    parts.append(mp)
                nc.vector.tensor_add(negm[:], parts[0][:], parts[1][:])
                nc.vector.tensor_add(negm[:], negm[:], parts[2][:])
                nc.vector.tensor_add(negm[:], negm[:], parts[3][:])

            # ---- main loop (xT pool opens after P1 pools closed)
            with tc.tile_pool(name="xT", bufs=1) as xTp:
                xT = {}
                for k in range(NK):
                    for h in range(2):
                        th = xTp.tile([P, HB], BF16, tag=f"xh{k}h{h}",
                                      name=f"xTh{k}_{h}")
                        nc.sync.dma_start_transpose(
                            th[:], xhi_d[h][:, k * P:(k + 1) * P])
                        tl = xTp.tile([P, HB], BF16, tag=f"xl{k}h{h}",
                                      name=f"xTl{k}_{h}")
                        nc.sync.dma_start_transpose(
                            tl[:], xlo_d[h][:, k * P:(k + 1) * P])
                        xT[("h", k, h)] = th
                        xT[("l", k, h)] = tl

                for o in range(NO):
                    wT = wTp.tile([P, NK, P], BF16, tag="wT", name=f"wT{o}")
                    nc.sync.dma_start_transpose(wT[:], wb_tiles[o][:, :])
                    psum = ps.tile([P, BS], F32, tag="acc", name=f"acc{o}")
                    for k in range(NK):
                        lhsT = wT[:, k, :]
                        nc.tensor.matmul(psum[:, 0:HB], lhsT, xT[("h", k, 0)][:],
                                         start=(k == 0), stop=False)
                        nc.tensor.matmul(psum[:, HB:BS], lhsT, xT[("h", k, 1)][:],
                                         start=(k == 0), stop=False)
                        nc.tensor.matmul(psum[:, 0:HB], lhsT, xT[("l", k, 0)][:],
                                         start=False, stop=(k == NK - 1))
                        nc.tensor.matmul(psum[:, HB:BS], lhsT, xT[("l", k, 1)][:],
                                         start=False, stop=(k == NK - 1))
                    ys = epi.tile([P, BS], BF16, tag="ys", name=f"ys{o}")
                    nc.scalar.activation(out=ys[:], in_=psum[:],
                                         func=mybir.ActivationFunctionType.Sign,
                                         bias=negm_tiles[o][:], scale=1.0)
                    ysT = epi.tile([P, NB, P], BF16, tag="ysT", name=f"ysT{o}")
                    nc.sync.dma_start_transpose(ysT[:], ys[:])
                    ysT32 = epi.tile([P, NB, P], F32, tag="ysT32", name=f"ysT32{o}")
                    nc.vector.tensor_copy(ysT32[:], ysT[:])
                    nc.sync.dma_start(
                        out[:, o * P:(o + 1) * P].rearrange("(t p) j -> p t j", p=P),
                        ysT32[:])

    nc.finalize()
    return nc


def _get_nc():
    global _CACHED_NC
    if _CACHED_NC is None:
        _CACHED_NC = _build_nc()
    return _CACHED_NC


def _run(x, W, **kw):
    nc = _get_nc()
    in_maps = [{"xs": x[c * BS:(c + 1) * BS], "W": W} for c in range(N_CORES)]
    res = run_bass_kernel_spmd(nc, in_maps, list(range(N_CORES)), **kw)
    full = np.concatenate([res.results[c]["out"] for c in range(N_CORES)], axis=0)
    return full, res


def kernel(x, W, b, gamma, beta):
    x = np.ascontiguousarray(x, dtype=np.float32)
    W = np.ascontiguousarray(W, dtype=np.float32)
    assert x.shape == (B, D) and W.shape == (D, D)
    if not (np.all(np.asarray(gamma) > 0) and np.all(np.asarray(beta) == 0)):
        # The sign(y - mean) reduction needs gamma > 0 and beta == 0 (always true for
        # this problem's inputs).  Otherwise fall back to a host computation.
        Wb = np.sign(W)
        y = x @ Wb.T + np.asarray(b, np.float32)
        mean = y.mean(0)
        var = ((y - mean) ** 2).mean(0)
        yn = (y - mean) / np.sqrt(var + 1e-5) * np.asarray(gamma) + np.asarray(beta)
        return np.sign(yn).astype(np.float32)
    full, _ = _run(x, W)
    return full.astype(np.float32, copy=False)
